# revision 1
# baseline (speedup 1.0000x reference)
"""Trainium2 Bass kernel: batched polynomial + Fourier-series point evaluator.

Math: for each point n and each of B=4 times t_b:
    y_poly[b, n]    = sum_{i<4}  poly[n, i] * t_b^i
    y_fourier[b, n] = sum_{k<18} fa[n, k]*cos(w_k t_b) + fb[n, k]*sin(w_k t_b)
(with Fourier bands gated by model_stage).

Because B=4 is tiny, both outputs are one linear map applied to the 40
per-point coefficients:  Y[:, n] = Basis.T @ W[n, :]  with Basis [40, 8]
computed on host (the transcendentals depend only on the 4 scalar times).
The device kernel is a pure streaming matmul over the coefficient tables.

Per-core layout (points sharded 8 ways, ~2^18 points/core, padded to
NP = 3*C so the contraction dim packs GROUPS=3 point-groups of 40 coeffs
= K=120):
  - host packs coefficients as fp16 [120, C]; each matmul column carries
    3 points, so one N=512 matmul evaluates 1536 points.
  - 4 matmuls per PSUM bank at tile_position (0, 32j) run concurrently on
    disjoint 32-column strips of the PE array (same [120, 32] stationary
    basis), filling a [128, 512] bank with 6144 points' outputs.
  - PSUM -> SBUF copies cast to fp16; row 32j+8g+jj then holds output jj
    (0-3 poly batch, 4-7 fourier batch) of group g on a contiguous point
    run, so output DMAs are plain 2D slices.
"""

import json

import numpy as np

import concourse.bass as bass
import concourse.mybir as mybir
import concourse.tile as tile
from concourse.bass_utils import run_bass_kernel_spmd

# Problem constants (hardcoded per harness contract).
B = 4
N_POINTS = 128 ** 3            # 2097152
N_CORES = 8
NC = N_POINTS // N_CORES       # 262144 real points per core
KH = 18                        # harmonics
NCOEF = 40                     # 4 poly + 18 cos + 18 sin

GROUPS = 3                     # point-groups stacked in contraction dim (K=120)
JT = 4                         # concurrent col-strip matmuls per PSUM bank
MM_N = 512                     # matmul moving free size (one PSUM bank of fp32)
SPANS = 2
U = 22                         # PSUM bank fills per span
BANK_COLS = JT * MM_N          # 2048 table columns per bank fill
C = SPANS * U * BANK_COLS      # 90112 table columns per core
NP = GROUPS * C                # 270336 padded points per core

_CACHED_NC = None
LAST_RESULTS = None            # BassKernelResults of the most recent run


def _build_module():
    nc = bass.Bass()
    dt = mybir.dt

    # Tiled layouts: every DMA moves one fully contiguous DRAM block.
    table = nc.dram_tensor(
        "table", [C // (2 * BANK_COLS), GROUPS * NCOEF, 2 * BANK_COLS],
        dt.float16, kind="ExternalInput")
    basis = nc.dram_tensor("basis", [GROUPS * NCOEF, 32], dt.float16,
                           kind="ExternalInput")
    out_t = nc.dram_tensor(
        "out_t", [SPANS, 2, JT, GROUPS, 8, U * MM_N // 2],
        dt.float16, kind="ExternalOutput")

    with tile.TileContext(nc) as tc:
        with (
            tc.tile_pool(name="const", bufs=1) as cpool,
            tc.tile_pool(name="inp", bufs=8) as ipool,
            tc.tile_pool(name="psum", bufs=8, space="PSUM") as ppool,
            tc.tile_pool(name="outp", bufs=3) as opool,
        ):
            basis_sb = cpool.tile([GROUPS * NCOEF, 32], dt.float16)
            nc.sync.dma_start(basis_sb[:, :], basis[:, :])

            HU = U // 2
            for span in range(SPANS):
                out_tile = opool.tile([128, U * MM_N], dt.float16)
                for it in range(HU):
                    # One in-DMA covers two bank fills; alternate HWDGE (SP)
                    # and SWDGE (Pool) so dispatch overheads run in parallel.
                    in_tile = ipool.tile(
                        [GROUPS * NCOEF, 2 * BANK_COLS], dt.float16
                    )
                    eng = nc.sync if it % 2 == 0 else nc.gpsimd
                    eng.dma_start(in_tile[:, :], table[span * HU + it])
                    for h in range(2):
                        u = 2 * it + h
                        ps = ppool.tile([128, MM_N], dt.float32)
                        for j in range(JT):
                            nc.tensor.matmul(
                                ps[32 * j : 32 * (j + 1), :],
                                basis_sb[:, :],
                                in_tile[:, h * BANK_COLS + MM_N * j
                                        : h * BANK_COLS + MM_N * (j + 1)],
                                start=True,
                                stop=True,
                                tile_position=(0, 32 * j),
                            )
                        nc.vector.tensor_copy(
                            out_tile[:, MM_N * u : MM_N * (u + 1)], ps[:, :]
                        )
                    # After the first half of the span's banks are cast,
                    # drain that half's rows early (finer out-DMAs overlap
                    # compute instead of bursting at the end).
                    if it == (U // 2 + 1) // 2 or it == HU - 1:
                        half = 0 if it == (U // 2 + 1) // 2 else 1
                        w0 = half * (U // 2) * MM_N
                        w1 = w0 + (U // 2) * MM_N
                        for j in range(JT):
                            for g in range(GROUPS):
                                row0 = 32 * j + 8 * g
                                nc.scalar.dma_start(
                                    out_t[span, half, j, g],
                                    out_tile[row0 : row0 + 8, w0:w1],
                                )
    return nc


def _dedupe_ldweights(m: dict) -> None:
    """Drop Ldweights instructions that reload the exact same stationary
    operand into the same PE array position as the previously retained one
    (the weights are static in this kernel).  Any waits on a dropped
    Ldweights migrate to the next instruction in the same engine stream."""
    def sig(ins):
        return json.dumps(
            {k: ins.get(k) for k in ("ins", "tile_position", "perf_mode",
                                     "is_transpose", "tile_size")},
            sort_keys=True,
        )

    def fix_block(b):
        last_by_pos = {}
        out = []
        pending_waits = []
        for ins in b.get("instructions", []):
            if ins.get("opcode") == "Ldweights":
                pos = tuple(ins.get("tile_position") or (0, 0))
                s = sig(ins)
                upd = (ins.get("sync_info") or {}).get("on_update", [])
                if last_by_pos.get(pos) == s and not upd:
                    pending_waits.extend(
                        (ins.get("sync_info") or {}).get("on_wait", []))
                    continue
                last_by_pos[pos] = s
            elif pending_waits and ins.get("engine") == "PE":
                si = ins.setdefault("sync_info", {"on_update": [], "on_wait": []})
                si["on_wait"] = pending_waits + si.get("on_wait", [])
                pending_waits = []
            out.append(ins)
        assert not pending_waits
        b["instructions"] = out
        for ch in b.get("blocks", []):
            fix_block(ch)

    for fn in m["functions"]:
        for b in fn.get("blocks", []):
            fix_block(b)


def _legalize_single_wait(bir_bytes: bytes) -> bytes:
    """Split multi-wait instructions: this walrus build's codegen accepts at
    most ONE sync-wait per ISA instruction.  Hoist all but the last wait onto
    NoOps inserted just before the instruction on the same engine stream
    (the sequencer executes them in order, so semantics are preserved)."""
    m = json.loads(bir_bytes)
    _dedupe_ldweights(m)
    n_split = 0

    def fix_block(b):
        nonlocal n_split
        out = []
        for ins in b.get("instructions", []):
            si = ins.get("sync_info")
            waits = (si or {}).get("on_wait", [])
            if len(waits) > 1 and ins.get("engine", "Unassigned") != "Unassigned":
                for w in waits[:-1]:
                    n_split += 1
                    out.append({
                        "debug": ins.get("debug", 0),
                        "engine": ins["engine"],
                        "ins": [],
                        "name": f"{ins['name']}-wsplit{n_split}",
                        "opcode": "NoOp",
                        "outs": [],
                        "sync_info": {"on_update": [], "on_wait": [w]},
                    })
                si["on_wait"] = [waits[-1]]
            out.append(ins)
        b["instructions"] = out
        for ch in b.get("blocks", []):
            fix_block(ch)

    for fn in m["functions"]:
        for b in fn.get("blocks", []):
            fix_block(b)
    return json.dumps(m).encode()


def _get_module():
    global _CACHED_NC
    if _CACHED_NC is None:
        nc = _build_module()
        orig = nc.to_json_bytes
        nc.to_json_bytes = lambda: _legalize_single_wait(orig())
        _CACHED_NC = nc
    return _CACHED_NC


def _host_basis(input_t: np.ndarray, model_stage) -> np.ndarray:
    """Packed stationary weights [120, 32] fp16: col 8g+jj = output jj of
    point-group g (jj 0-3 poly batch, 4-7 fourier batch)."""
    stage = int(model_stage)
    curr = min(stage, 3) if stage >= 0 else 3
    mask = np.zeros(KH, dtype=np.float64)
    for s, e, req in ((0, 3, 1), (3, 9, 2), (9, KH, 3)):
        if curr >= req:
            mask[s:e] = 1.0

    t = np.asarray(input_t, dtype=np.float64)
    Vp = np.stack([t ** i for i in range(4)], axis=0)           # [4, B]
    w = 2.0 * np.pi * np.arange(1, KH + 1, dtype=np.float64)    # [18]
    Cc = np.cos(np.outer(w, t)) * mask[:, None]                 # [18, B]
    Ss = np.sin(np.outer(w, t)) * mask[:, None]                 # [18, B]

    B8 = np.zeros((NCOEF, 8), dtype=np.float64)
    B8[0:4, 0:4] = Vp
    B8[4:22, 4:8] = Cc
    B8[22:40, 4:8] = Ss

    basis = np.zeros((GROUPS * NCOEF, 32), dtype=np.float64)
    for g in range(GROUPS):
        basis[NCOEF * g : NCOEF * (g + 1), 8 * g : 8 * g + 8] = B8
    return basis.astype(np.float16)


def kernel(input_t, poly_coeffs, fourier_a, fourier_b, model_stage):
    global LAST_RESULTS
    input_t = np.asarray(input_t, dtype=np.float32)
    poly_coeffs = np.asarray(poly_coeffs, dtype=np.float32)
    fourier_a = np.asarray(fourier_a, dtype=np.float32)
    fourier_b = np.asarray(fourier_b, dtype=np.float32)
    assert input_t.shape == (B,)
    assert poly_coeffs.shape == (N_POINTS, 4)
    assert fourier_a.shape == (N_POINTS, KH)
    assert fourier_b.shape == (N_POINTS, KH)

    basis = _host_basis(input_t, model_stage)

    # Pack per-core tables [120, C] fp16 with the device's column order:
    # table col (span*U + u)*BANK_COLS + j*MM_N + f  holds point
    # g*C + span*U*BANK_COLS + j*U*MM_N + u*MM_N + f   (j <-> u swapped so
    # each output row covers a contiguous DRAM run).
    W = np.concatenate([poly_coeffs, fourier_a, fourier_b], axis=1)
    W = W.astype(np.float16)                                    # [N, 40]
    Wp = np.zeros((N_CORES, NP, NCOEF), dtype=np.float16)
    Wp[:, :NC] = W.reshape(N_CORES, NC, NCOEF)
    Wp = Wp.reshape(N_CORES, GROUPS, SPANS, JT, U, MM_N, NCOEF)
    Wp = Wp.transpose(0, 1, 6, 2, 4, 3, 5)   # core, g, k, span, u, j, f
    tables = np.ascontiguousarray(Wp).reshape(N_CORES, GROUPS * NCOEF, C)
    # Tile the column axis so each in-DMA reads one contiguous DRAM block.
    NT = C // (2 * BANK_COLS)
    tables = np.ascontiguousarray(
        tables.reshape(N_CORES, GROUPS * NCOEF, NT, 2 * BANK_COLS)
        .transpose(0, 2, 1, 3))

    nc = _get_module()
    in_maps = [{"table": tables[c], "basis": basis} for c in range(N_CORES)]
    LAST_RESULTS = run_bass_kernel_spmd(nc, in_maps, core_ids=list(range(N_CORES)))
    results = LAST_RESULTS.results

    outs = []
    for r in results:
        ot = r["out_t"]  # [SPANS, 2, JT, GROUPS, 8, U*MM_N/2]
        o8 = ot.transpose(4, 3, 0, 2, 1, 5).reshape(8, NP)
        outs.append(o8[:, :NC].astype(np.float32))
    out = np.concatenate(outs, axis=1)
    return out[0:4], out[4:8]



# revision 2
# speedup vs baseline: 1.1646x; 1.1646x over previous
"""Trainium2 Bass kernel: batched polynomial + Fourier-series point evaluator.

Math: for each point n and each of B=4 times t_b:
    y_poly[b, n]    = sum_{i<4}  poly[n, i] * t_b^i
    y_fourier[b, n] = sum_{k<18} fa[n, k]*cos(w_k t_b) + fb[n, k]*sin(w_k t_b)
(with Fourier bands gated by model_stage).

Because B=4 is tiny, both outputs are one linear map applied to the 40
per-point coefficients:  Y[:, n] = Basis.T @ W[n, :]  with Basis [40, 8]
computed on host (the transcendentals depend only on the 4 scalar times).
The device kernel is a pure streaming matmul over the coefficient tables;
it is DMA-bound, so the layout is chosen for DMA descriptor efficiency:

  - table DRAM [120, COLS] fp16, row 40g+i = coeff i of point-group g.
    Every in-DMA slices a column range -> one contiguous 4-32 KB
    descriptor per partition (large descriptors amortize the ~300 ns
    per-descriptor HBM latency that capped the old 8 KB layout at
    ~13.5 GB/s per SDMA engine).
  - in-DMA chunk sizes ramp 1,2,4,8.. banks so compute starts early but
    steady-state descriptors are 32 KB.
  - 4 matmuls per PSUM bank at tile_position (0, 32j) on disjoint
    32-column strips of the PE array (same [120, 32] stationary basis).
  - PSUM -> SBUF copies cast to fp16 into a [128, cols] out tile; out-DMAs
    write the full 128-partition tile (all 16 SDMA engines) instead of
    8-row slices (2 engines), which previously serialized a ~40 us tail.
"""

import json

import numpy as np

import concourse.bass as bass
import concourse.mybir as mybir
import concourse.tile as tile
from concourse.bass_utils import run_bass_kernel_spmd

# Problem constants (hardcoded per harness contract).
B = 4
N_POINTS = 128 ** 3            # 2097152
N_CORES = 8
NC_PTS = N_POINTS // N_CORES   # 262144 real points per core
KH = 18                        # harmonics
NCOEF = 40                     # 4 poly + 18 cos + 18 sin

GROUPS = 3                     # point-groups stacked in contraction dim (K=120)
JT = 4                         # concurrent col-strip matmuls per PSUM bank
MM_N = 512                     # matmul moving free size (one PSUM bank of fp32)
BANKS = 43                     # PSUM-bank fills per core (43*2048*3 = 264192 pts)
BCOLS = JT * MM_N              # 2048 table columns per bank
COLS = BANKS * BCOLS           # 88064 table columns per core
NP = GROUPS * COLS             # 264192 padded points per core

IN_CHUNKS = (1, 2, 4, 8, 8, 8, 8, 4)   # banks per in-DMA (sum = 43)
OUT_CHUNKS = (16, 16, 8, 2, 1)         # banks per out-DMA (sum = 43)

_CACHED_NC = None
LAST_RESULTS = None            # BassKernelResults of the most recent run


def _build_module():
    nc = bass.Bass()
    dt = mybir.dt

    table = nc.dram_tensor("table", [GROUPS * NCOEF, COLS], dt.float16,
                           kind="ExternalInput")
    basis = nc.dram_tensor("basis", [GROUPS * NCOEF, 32], dt.float16,
                           kind="ExternalInput")
    out_t = nc.dram_tensor("out_t", [128, BANKS * MM_N], dt.float16,
                           kind="ExternalOutput")

    with tile.TileContext(nc) as tc:
        with (
            tc.tile_pool(name="const", bufs=1) as cpool,
            tc.tile_pool(name="inp", bufs=3) as ipool,
            tc.tile_pool(name="psum", bufs=8, space="PSUM") as ppool,
            tc.tile_pool(name="outp", bufs=2) as opool,
        ):
            basis_sb = cpool.tile([GROUPS * NCOEF, 32], dt.float16)
            nc.sync.dma_start(basis_sb[:, :], basis[:, :])

            out_tile = None
            oc = 0                 # index into OUT_CHUNKS
            ob0 = 0                # first bank of current out tile
            gb = 0                 # global bank index
            for ci, nb in enumerate(IN_CHUNKS):
                in_tile = ipool.tile([GROUPS * NCOEF, nb * BCOLS], dt.float16)
                eng = nc.sync if ci % 2 == 0 else nc.gpsimd
                eng.dma_start(in_tile[:, :],
                              table[:, gb * BCOLS : (gb + nb) * BCOLS])
                for b in range(nb):
                    if out_tile is None:
                        onb = OUT_CHUNKS[oc]
                        out_tile = opool.tile([128, onb * MM_N], dt.float16)
                        ob0 = gb
                    ps = ppool.tile([128, MM_N], dt.float32)
                    for j in range(JT):
                        nc.tensor.matmul(
                            ps[32 * j : 32 * (j + 1), :],
                            basis_sb[:, :],
                            in_tile[:, b * BCOLS + MM_N * j
                                    : b * BCOLS + MM_N * (j + 1)],
                            start=True,
                            stop=True,
                            tile_position=(0, 32 * j),
                        )
                    w0 = (gb - ob0) * MM_N
                    nc.vector.tensor_copy(
                        out_tile[:, w0 : w0 + MM_N], ps[:, :]
                    )
                    gb += 1
                    if gb - ob0 == OUT_CHUNKS[oc]:
                        nc.scalar.dma_start(
                            out_t[:, ob0 * MM_N : gb * MM_N], out_tile[:, :]
                        )
                        out_tile = None
                        oc += 1
    return nc


def _dedupe_ldweights(m: dict) -> None:
    """Drop Ldweights instructions that reload the exact same stationary
    operand into the same PE array position as the previously retained one
    (the weights are static in this kernel).  Any waits on a dropped
    Ldweights migrate to the next instruction in the same engine stream."""
    def sig(ins):
        return json.dumps(
            {k: ins.get(k) for k in ("ins", "tile_position", "perf_mode",
                                     "is_transpose", "tile_size")},
            sort_keys=True,
        )

    def fix_block(b):
        last_by_pos = {}
        out = []
        pending_waits = []
        for ins in b.get("instructions", []):
            if ins.get("opcode") == "Ldweights":
                pos = tuple(ins.get("tile_position") or (0, 0))
                s = sig(ins)
                upd = (ins.get("sync_info") or {}).get("on_update", [])
                if last_by_pos.get(pos) == s and not upd:
                    pending_waits.extend(
                        (ins.get("sync_info") or {}).get("on_wait", []))
                    continue
                last_by_pos[pos] = s
            elif pending_waits and ins.get("engine") == "PE":
                si = ins.setdefault("sync_info", {"on_update": [], "on_wait": []})
                si["on_wait"] = pending_waits + si.get("on_wait", [])
                pending_waits = []
            out.append(ins)
        assert not pending_waits
        b["instructions"] = out
        for ch in b.get("blocks", []):
            fix_block(ch)

    for fn in m["functions"]:
        for b in fn.get("blocks", []):
            fix_block(b)


def _legalize_single_wait(bir_bytes: bytes) -> bytes:
    """Split multi-wait instructions: this walrus build's codegen accepts at
    most ONE sync-wait per ISA instruction.  Hoist all but the last wait onto
    NoOps inserted just before the instruction on the same engine stream
    (the sequencer executes them in order, so semantics are preserved)."""
    m = json.loads(bir_bytes)
    _dedupe_ldweights(m)
    n_split = 0

    def fix_block(b):
        nonlocal n_split
        out = []
        for ins in b.get("instructions", []):
            si = ins.get("sync_info")
            waits = (si or {}).get("on_wait", [])
            if len(waits) > 1 and ins.get("engine", "Unassigned") != "Unassigned":
                for w in waits[:-1]:
                    n_split += 1
                    out.append({
                        "debug": ins.get("debug", 0),
                        "engine": ins["engine"],
                        "ins": [],
                        "name": f"{ins['name']}-wsplit{n_split}",
                        "opcode": "NoOp",
                        "outs": [],
                        "sync_info": {"on_update": [], "on_wait": [w]},
                    })
                si["on_wait"] = [waits[-1]]
            out.append(ins)
        b["instructions"] = out
        for ch in b.get("blocks", []):
            fix_block(ch)

    for fn in m["functions"]:
        for b in fn.get("blocks", []):
            fix_block(b)
    return json.dumps(m).encode()


def _get_module():
    global _CACHED_NC
    if _CACHED_NC is None:
        nc = _build_module()
        orig = nc.to_json_bytes
        nc.to_json_bytes = lambda: _legalize_single_wait(orig())
        _CACHED_NC = nc
    return _CACHED_NC


def _host_basis(input_t: np.ndarray, model_stage) -> np.ndarray:
    """Packed stationary weights [120, 32] fp16: col 8g+jj = output jj of
    point-group g (jj 0-3 poly batch, 4-7 fourier batch)."""
    stage = int(model_stage)
    curr = min(stage, 3) if stage >= 0 else 3
    mask = np.zeros(KH, dtype=np.float64)
    for s, e, req in ((0, 3, 1), (3, 9, 2), (9, KH, 3)):
        if curr >= req:
            mask[s:e] = 1.0

    t = np.asarray(input_t, dtype=np.float64)
    Vp = np.stack([t ** i for i in range(4)], axis=0)           # [4, B]
    w = 2.0 * np.pi * np.arange(1, KH + 1, dtype=np.float64)    # [18]
    Cc = np.cos(np.outer(w, t)) * mask[:, None]                 # [18, B]
    Ss = np.sin(np.outer(w, t)) * mask[:, None]                 # [18, B]

    B8 = np.zeros((NCOEF, 8), dtype=np.float64)
    B8[0:4, 0:4] = Vp
    B8[4:22, 4:8] = Cc
    B8[22:40, 4:8] = Ss

    basis = np.zeros((GROUPS * NCOEF, 32), dtype=np.float64)
    for g in range(GROUPS):
        basis[NCOEF * g : NCOEF * (g + 1), 8 * g : 8 * g + 8] = B8
    return basis.astype(np.float16)


def kernel(input_t, poly_coeffs, fourier_a, fourier_b, model_stage):
    global LAST_RESULTS
    input_t = np.asarray(input_t, dtype=np.float32)
    poly_coeffs = np.asarray(poly_coeffs, dtype=np.float32)
    fourier_a = np.asarray(fourier_a, dtype=np.float32)
    fourier_b = np.asarray(fourier_b, dtype=np.float32)
    assert input_t.shape == (B,)
    assert poly_coeffs.shape == (N_POINTS, 4)
    assert fourier_a.shape == (N_POINTS, KH)
    assert fourier_b.shape == (N_POINTS, KH)

    basis = _host_basis(input_t, model_stage)

    # Per-core table [120, COLS] fp16: row 40g+i, col c holds coeff i of
    # point g*COLS + c.
    W = np.concatenate([poly_coeffs, fourier_a, fourier_b], axis=1)
    W = W.astype(np.float16)                                    # [N, 40]
    Wp = np.zeros((N_CORES, NP, NCOEF), dtype=np.float16)
    Wp[:, :NC_PTS] = W.reshape(N_CORES, NC_PTS, NCOEF)
    tables = np.ascontiguousarray(
        Wp.reshape(N_CORES, GROUPS, COLS, NCOEF).transpose(0, 1, 3, 2)
    ).reshape(N_CORES, GROUPS * NCOEF, COLS)

    nc = _get_module()
    in_maps = [{"table": tables[c], "basis": basis} for c in range(N_CORES)]
    LAST_RESULTS = run_bass_kernel_spmd(nc, in_maps, core_ids=list(range(N_CORES)))
    results = LAST_RESULTS.results

    outs = []
    for r in results:
        ot = r["out_t"]  # [128, BANKS*512]; row 32j+8g+jj, col 512u+f
        o = ot.reshape(JT, 4, 8, BANKS, MM_N)[:, :GROUPS]  # [j, g, jj, u, f]
        o = o.transpose(2, 1, 3, 0, 4)                     # [jj, g, u, j, f]
        outs.append(o.reshape(8, NP)[:, :NC_PTS].astype(np.float32))
    out = np.concatenate(outs, axis=1)
    return out[0:4], out[4:8]
